# revision 8
# baseline (speedup 1.0000x reference)
"""Trainium2 Bass kernel for nn_CrossAttention (b=4, n=2048, j=2048, h=8, d=64).

Sharding: 8 cores = (batch 4) x (query-half 2). Each core computes all 8 heads
for 1024 query rows of one batch; context/k/v work is duplicated across the two
cores of a batch. No collectives; gather is pure concatenation.

Key optimizations over the naive version:
 - Host-side layout prep (zero FLOPs): x and ctx are pre-transposed in numpy,
   eliminating all on-device PE transposes; context rows are PACKED by mask
   (masked rows contribute exactly zero attention weight, so they are dropped
   and the j extent shrinks from 2048 to round128(max unmasked count), ~1152
   for a ~50% dense mask). Padded slots get bias -30 -> exp ~ 1e-13 ~ 0.
 - Software pipelining: S/exp of head h overlaps AV of head h-1 on the PE,
   and the Q/K projections for later head-pairs are interleaved into the
   attention loop so the scalar engine (exp) starts ~early and never gates.

Per-core pipeline (all matmuls fp16, out proj fp32r):
  qT = Wq^T @ xT  [inner, n]     kT = Wk^T @ ctxT  [inner, jP]
  v  = ctxT^T @ Wv -> vaug f16 [jP, h, d+1] (ones col => denominator row)
  per head: S[j128, n] = kT_h^T @ qT_h   (K=64, head pairs row-tiled)
            pt = exp(0.125*S + maskbias) (ACT, bf16)
            avp[d+1, n] = vaug_h^T @ pt  (accum over j)  -> row d = denom l
            oT_h = avp[0:64] * broadcast(1/l)
  out = oT^T @ Wo + b_o  -> DMA
"""
import numpy as np
from contextlib import ExitStack

from concourse import bacc, mybir, tile
from concourse.bass_utils import run_bass_kernel_spmd

F32 = mybir.dt.float32
F32R = mybir.dt.float32r
BF16 = mybir.dt.bfloat16
F16 = mybir.dt.float16

HEADS = 8
D = 64
N_CORE = 1024   # query rows per core
CQ = 1024       # query_dim
CK = 768        # context_dim
INNER = 512
OUT = 1024
P = 128
SCALE = 0.125
MASK_NEG = -30.0

KQ = CQ // P          # 8
KC = CK // P          # 6
NB = N_CORE // P      # 8
DB = INNER // P       # 4
NG = N_CORE // 512    # 2


def build_nc(jbt):
    J = jbt * P
    nc = bacc.Bacc("TRN2", target_bir_lowering=False)
    xT_d = nc.dram_tensor("x", [CQ, N_CORE], F16, kind="ExternalInput")
    ctxT_d = nc.dram_tensor("ctx", [CK, J], F16, kind="ExternalInput")
    mb_d = nc.dram_tensor("mb", [J, 1], F32, kind="ExternalInput")
    wq_d = nc.dram_tensor("wq", [CQ, INNER], F16, kind="ExternalInput")
    wk_d = nc.dram_tensor("wk", [CK, INNER], F16, kind="ExternalInput")
    wv_d = nc.dram_tensor("wv", [CK, INNER], F16, kind="ExternalInput")
    wo_d = nc.dram_tensor("wo", [INNER, OUT], F32, kind="ExternalInput")
    bo_d = nc.dram_tensor("bo", [1, OUT], F32, kind="ExternalInput")
    out_d = nc.dram_tensor("out", [N_CORE, OUT], F32, kind="ExternalOutput")

    # K-proj j-groups (<=512 each)
    jgs = []
    off = 0
    while off < J:
        jl = min(512, J - off)
        jgs.append((off, jl))
        off += jl

    with ExitStack() as top:
        tc = top.enter_context(tile.TileContext(nc))
        consts = top.enter_context(tc.tile_pool(name="consts", bufs=1))
        mb_sb = consts.tile([P, jbt], F32)
        bo_sb = consts.tile([1, OUT], F32)
        b_bc = consts.tile([P, OUT], F32)

        persist = top.enter_context(tc.tile_pool(name="persist", bufs=1))
        xT = persist.tile([P, KQ, N_CORE], F16, name="xT")
        ctxT = persist.tile([P, KC, J], F16, name="ctxT")
        wq_sb = persist.tile([P, KQ, INNER], F16, name="wq")
        wk_sb = persist.tile([P, KC, INNER], F16, name="wk")
        wv_sb = persist.tile([P, KC, INNER], F16, name="wv")
        wo_sb = persist.tile([P, DB, OUT], F32R, name="wo")
        qT = persist.tile([P, DB, N_CORE], F16, name="qT")
        kT = persist.tile([P, DB, J], F16, name="kT")
        vaug = persist.tile([P, jbt, HEADS, D + 1], F16, name="vaug")
        oT = persist.tile([P, DB, N_CORE], F32R, name="oT")

        ptp = top.enter_context(tc.tile_pool(name="ptp", bufs=3))
        ps_s = top.enter_context(tc.tile_pool(name="ps_s", bufs=2, space="PSUM"))
        ps_av = top.enter_context(tc.tile_pool(name="ps_av", bufs=2, space="PSUM"))
        ps_p = top.enter_context(tc.tile_pool(name="ps_p", bufs=2, space="PSUM"))
        small = top.enter_context(tc.tile_pool(name="small", bufs=2))
        outp = top.enter_context(tc.tile_pool(name="outp", bufs=3))

        # ---------- input DMAs ----------
        # Two independent HWDGE queues (SP + Activation): Q-proj inputs on SP,
        # K-proj inputs on ACT so both critical streams flow in parallel.
        def dma_rearr(eng, dst, src, c0, c1, dtype=None):
            ap = src[c0 * P:c1 * P, :].rearrange("(c p) n -> p c n", p=P)
            if dtype is not None:
                ap = ap.bitcast(dtype)
            eng.dma_start(out=dst[:, c0:c1, :], in_=ap)

        dma_rearr(nc.sync, xT, xT_d, 0, 4)
        dma_rearr(nc.scalar, ctxT, ctxT_d, 0, 3)
        dma_rearr(nc.sync, wq_sb, wq_d, 0, KQ)
        dma_rearr(nc.scalar, wk_sb, wk_d, 0, KC)
        dma_rearr(nc.sync, xT, xT_d, 4, KQ)
        dma_rearr(nc.scalar, ctxT, ctxT_d, 3, KC)
        nc.scalar.dma_start(
            out=mb_sb, in_=mb_d.rearrange("(c p) o -> p (c o)", p=P)
        )
        dma_rearr(nc.sync, wv_sb, wv_d, 0, KC)
        dma_rearr(nc.sync, wo_sb, wo_d, 0, 2, dtype=F32R)
        dma_rearr(nc.sync, wo_sb, wo_d, 2, DB, dtype=F32R)
        nc.sync.dma_start(out=bo_sb, in_=bo_d[:, :])
        nc.gpsimd.partition_broadcast(b_bc, bo_sb)

        # ---------- projection units ----------
        def q_unit(db, ng):
            def emit():
                qp = ps_p.tile([P, 512], F32, name="pp")
                for kc in range(KQ):
                    nc.tensor.matmul(
                        qp,
                        wq_sb[:, kc, db * P:(db + 1) * P],
                        xT[:, kc, ng * 512:(ng + 1) * 512],
                        start=(kc == 0), stop=(kc == KQ - 1),
                    )
                nc.vector.tensor_copy(out=qT[:, db, ng * 512:(ng + 1) * 512], in_=qp)
            return emit

        def k_unit(db, j0, jl):
            def emit():
                kp = ps_p.tile([P, 512], F32, name="pp")
                for kc in range(KC):
                    nc.tensor.matmul(
                        kp[:, 0:jl],
                        wk_sb[:, kc, db * P:(db + 1) * P],
                        ctxT[:, kc, j0:j0 + jl],
                        start=(kc == 0), stop=(kc == KC - 1),
                    )
                nc.vector.tensor_copy(out=kT[:, db, j0:j0 + jl], in_=kp[:, 0:jl])
            return emit

        def v_unit(jb):
            def emit():
                vp = ps_p.tile([P, 512], F32, name="pp")
                for kc in range(KC):
                    nc.tensor.matmul(
                        vp,
                        ctxT[:, kc, jb * P:(jb + 1) * P],
                        wv_sb[:, kc, :],
                        start=(kc == 0), stop=(kc == KC - 1),
                    )
                nc.vector.tensor_copy(
                    out=vaug[:, jb, :, 0:D],
                    in_=vp.rearrange("p (h d) -> p h d", h=HEADS),
                )
                nc.vector.memset(vaug[:, jb, :, D:D + 1], 1.0)
            return emit

        # upfront: only what S(h0) needs; V-proj slides into head 0's loop
        # (AV(h0) runs during head 1, so all v_units are emitted before use)
        q_unit(0, 0)()
        q_unit(0, 1)()
        for (j0, jl) in jgs:
            k_unit(0, j0, jl)()

        # background units: V-proj first (popped 1/step during head 0),
        # then Q/K projections for head-pairs 1..3 (popped 1 per 3 steps)
        bg_v = [v_unit(jb) for jb in range(jbt)]
        bg = []
        for db in range(1, DB):
            bg.append(q_unit(db, 0))
            bg.append(q_unit(db, 1))
            for (j0, jl) in jgs:
                bg.append(k_unit(db, j0, jl))

        # ---------- attention ----------
        def kslice(h, jb):
            return kT[64 * (h % 2):64 * (h % 2) + 64, h // 2, jb * P:(jb + 1) * P]

        def qslice(h, ng):
            return qT[64 * (h % 2):64 * (h % 2) + 64, h // 2, ng * 512:(ng + 1) * 512]

        pts = {}

        def av_steps(h):
            """Yield closures: AV matmuls + normalize for head h (2 ng groups)."""
            for ng in range(NG):
                avp = ps_av.tile([D + 1, 512], F32, name="av")

                def mk_mm(jb, avp=avp, ng=ng):
                    def emit():
                        nc.tensor.matmul(
                            avp,
                            vaug[:, jb, h, :],
                            pts[h][:, jb, ng * 512:(ng + 1) * 512],
                            start=(jb == 0), stop=(jb == jbt - 1),
                        )
                    return emit

                for jb in range(jbt):
                    yield mk_mm(jb)

                def norm(avp=avp, ng=ng):
                    l_sb = small.tile([1, 512], F32, name="l_sb")
                    nc.vector.tensor_copy(out=l_sb, in_=avp[D:D + 1, :])
                    r_f = small.tile([1, 512], F32, name="r_f")
                    nc.vector.reciprocal_approx_fast(r_f, l_sb)
                    bc_sb = small.tile([D, 512], F32, name="bc_sb")
                    nc.gpsimd.partition_broadcast(bc_sb, r_f)
                    nc.vector.tensor_mul(
                        oT[64 * (h % 2):64 * (h % 2) + 64, h // 2,
                           ng * 512:(ng + 1) * 512],
                        avp[0:D, :],
                        bc_sb,
                    )
                yield norm

        for h in range(HEADS):
            pts[h] = ptp.tile([P, jbt, N_CORE], BF16, name="pt")
            av_it = iter(av_steps(h - 1)) if h >= 1 else None
            for jb in range(jbt):
                sp = ps_s.tile([P, N_CORE], F32, name="sp")
                for ng in range(NG):
                    nc.tensor.matmul(
                        sp[:, ng * 512:(ng + 1) * 512],
                        kslice(h, jb), qslice(h, ng),
                        start=True, stop=True,
                    )
                nc.scalar.activation(
                    out=pts[h][:, jb, :], in_=sp,
                    func=mybir.ActivationFunctionType.Exp,
                    bias=mb_sb[:, jb:jb + 1], scale=SCALE,
                )
                if av_it is not None:
                    for step in (next(av_it, None), next(av_it, None)):
                        if step is not None:
                            step()
                if bg_v:
                    bg_v.pop(0)()
                elif jb % 3 == 2 and bg:
                    bg.pop(0)()
            if av_it is not None:
                for step in av_it:
                    step()
        while bg_v:
            bg_v.pop(0)()
        while bg:
            bg.pop(0)()

        # ---------- tail: AV(h7) interleaved with out = oT^T @ Wo + b ------
        def o_group(nb, og):
            op = ps_p.tile([P, 512], F32, name="pp")
            for t in range(DB):
                nc.tensor.matmul(
                    op,
                    oT[:, t, nb * P:(nb + 1) * P],
                    wo_sb[:, t, og * 512:(og + 1) * 512],
                    start=(t == 0), stop=(t == DB - 1),
                )
            ob = outp.tile([P, 512], F32, name="ob")
            nc.vector.tensor_add(ob, op, b_bc[:, og * 512:(og + 1) * 512])
            eng = nc.sync if og == 0 else nc.scalar  # drain on 2 HWDGE queues
            eng.dma_start(
                out=out_d[nb * P:(nb + 1) * P, og * 512:(og + 1) * 512],
                in_=ob,
            )

        av7 = iter(av_steps(HEADS - 1))
        for _ in range(jbt + 1):     # ng0 matmuls + norm -> oT[.., 0:512]
            next(av7)()
        rest = list(av7)             # ng1 matmuls + norm
        ri = 0
        for nb in range(NB // 2):    # out rows 0-511 ready; overlap with ng1
            for og in range(OUT // 512):
                for _ in range(2):
                    if ri < len(rest):
                        rest[ri]()
                        ri += 1
                o_group(nb, og)
        while ri < len(rest):
            rest[ri]()
            ri += 1
        for nb in range(NB // 2, NB):
            for og in range(OUT // 512):
                o_group(nb, og)

    nc.finalize()
    return nc


_NC_CACHE = {}
_LAST_JBT = 9


def _get_nc(jbt=None):
    global _LAST_JBT
    if jbt is None:
        jbt = _LAST_JBT
    _LAST_JBT = jbt
    if jbt not in _NC_CACHE:
        _NC_CACHE[jbt] = build_nc(jbt)
    return _NC_CACHE[jbt]


def make_in_maps(x, context, mask, W_q, W_k, W_v, W_o, b_o):
    global _LAST_JBT
    x = np.asarray(x, dtype=np.float32)
    context = np.asarray(context, dtype=np.float32)
    mask = np.asarray(mask).astype(bool)
    b, n, _ = x.shape
    j_full = context.shape[1]

    counts = mask.sum(axis=1)
    jbt = max(1, int(-(-int(counts.max()) // P)))  # ceil
    jbt = min(jbt, j_full // P)
    _LAST_JBT = jbt
    J = jbt * P

    shared = {
        "wq": np.ascontiguousarray(np.asarray(W_q, dtype=np.float16)),
        "wk": np.ascontiguousarray(np.asarray(W_k, dtype=np.float16)),
        "wv": np.ascontiguousarray(np.asarray(W_v, dtype=np.float16)),
        "wo": np.ascontiguousarray(np.asarray(W_o, dtype=np.float32)),
        "bo": np.ascontiguousarray(
            np.asarray(b_o, dtype=np.float32).reshape(1, OUT)
        ),
    }
    # per-batch: pack unmasked context rows first (order-preserving), truncate
    # to J (dropped rows are all masked => contribute exactly 0), transpose.
    ctxT_b, mb_b = [], []
    for bi in range(b):
        idx = np.argsort(~mask[bi], kind="stable")[:J]
        ctxp = context[bi][idx]
        mkp = mask[bi][idx]
        ctxT_b.append(np.ascontiguousarray(ctxp.T.astype(np.float16)))
        mb_b.append(
            np.where(mkp, 0.0, MASK_NEG).astype(np.float32).reshape(J, 1)
        )

    in_maps = []
    for c in range(8):
        bi, nh = c // 2, c % 2
        xT_c = np.ascontiguousarray(
            x[bi, nh * N_CORE:(nh + 1) * N_CORE].T.astype(np.float16)
        )
        in_maps.append({
            "x": xT_c,
            "ctx": ctxT_b[bi],
            "mb": mb_b[bi],
            **shared,
        })
    return in_maps


def kernel(x, context, mask, W_q, W_k, W_v, W_o, b_o):
    in_maps = make_in_maps(x, context, mask, W_q, W_k, W_v, W_o, b_o)
    nc = _get_nc(_LAST_JBT)
    res = run_bass_kernel_spmd(nc, in_maps, core_ids=list(range(8)))
    out = np.empty((4, 2048, OUT), dtype=np.float32)
    for c in range(8):
        bi, nh = c // 2, c % 2
        out[bi, nh * N_CORE:(nh + 1) * N_CORE] = res.results[c]["out"]
    return out


# revision 12
# speedup vs baseline: 1.0456x; 1.0456x over previous
"""Trainium2 Bass kernel for nn_CrossAttention (b=4, n=2048, j=2048, h=8, d=64).

Sharding: 8 cores = (batch 4) x (query-half 2). Each core computes all 8 heads
for 1024 query rows of one batch; context/k/v work is duplicated across the two
cores of a batch. No collectives; gather is pure concatenation.

Key optimizations over the naive version:
 - Host-side layout prep (zero FLOPs): x and ctx are pre-transposed in numpy,
   eliminating all on-device PE transposes; context rows are PACKED by mask
   (masked rows contribute exactly zero attention weight, so they are dropped
   and the j extent shrinks from 2048 to round128(max unmasked count), ~1152
   for a ~50% dense mask). Padded slots get bias -30 -> exp ~ 1e-13 ~ 0.
 - Software pipelining: S/exp of head h overlaps AV of head h-1 on the PE,
   and the Q/K projections for later head-pairs are interleaved into the
   attention loop so the scalar engine (exp) starts ~early and never gates.

Per-core pipeline (all matmuls fp16, out proj fp32r):
  qT = Wq^T @ xT  [inner, n]     kT = Wk^T @ ctxT  [inner, jP]
  v  = ctxT^T @ Wv -> vaug f16 [jP, h, d+1] (ones col => denominator row)
  per head: S[j128, n] = kT_h^T @ qT_h   (K=64, head pairs row-tiled)
            pt = exp(0.125*S + maskbias) (ACT, bf16)
            avp[d+1, n] = vaug_h^T @ pt  (accum over j)  -> row d = denom l
            oT_h = avp[0:64] * broadcast(1/l)
  out = oT^T @ Wo + b_o  -> DMA
"""
import numpy as np
from contextlib import ExitStack

from concourse import bacc, mybir, tile
from concourse.bass_utils import run_bass_kernel_spmd

F32 = mybir.dt.float32
F32R = mybir.dt.float32r
BF16 = mybir.dt.bfloat16
F16 = mybir.dt.float16

HEADS = 8
D = 64
N_CORE = 1024   # query rows per core
CQ = 1024       # query_dim
CK = 768        # context_dim
INNER = 512
OUT = 1024
P = 128
SCALE = 0.125
MASK_NEG = -30.0

KQ = CQ // P          # 8
KC = CK // P          # 6
NB = N_CORE // P      # 8
DB = INNER // P       # 4
NG = N_CORE // 512    # 2


def build_nc(jbt):
    J = jbt * P
    nc = bacc.Bacc("TRN2", target_bir_lowering=False)
    # All inputs host-swizzled to partition-major [128, ...] layouts so every
    # DMA is a plain 2D transfer (128 large contiguous runs, cheap desc-gen).
    xT_d = nc.dram_tensor("x", [P, KQ * N_CORE], F16, kind="ExternalInput")
    ctxT_d = nc.dram_tensor("ctx", [P, KC * J], F16, kind="ExternalInput")
    mb_d = nc.dram_tensor("mb", [P, jbt], F32, kind="ExternalInput")
    wq0_d = nc.dram_tensor("wq0", [P, KQ * P], F16, kind="ExternalInput")
    wqr_d = nc.dram_tensor("wqr", [P, KQ * 3 * P], F16, kind="ExternalInput")
    wk0_d = nc.dram_tensor("wk0", [P, KC * P], F16, kind="ExternalInput")
    wkr_d = nc.dram_tensor("wkr", [P, KC * 3 * P], F16, kind="ExternalInput")
    wv_d = nc.dram_tensor("wv", [P, KC * INNER], F16, kind="ExternalInput")
    wo_d = nc.dram_tensor("wo", [P, DB * OUT], F32, kind="ExternalInput")
    bo_d = nc.dram_tensor("bo", [1, OUT], F32, kind="ExternalInput")
    out_d = nc.dram_tensor("out", [N_CORE, OUT], F32, kind="ExternalOutput")

    # K-proj j-groups (<=512 each)
    jgs = []
    off = 0
    while off < J:
        jl = min(512, J - off)
        jgs.append((off, jl))
        off += jl

    with ExitStack() as top:
        tc = top.enter_context(tile.TileContext(nc))
        consts = top.enter_context(tc.tile_pool(name="consts", bufs=1))
        mb_sb = consts.tile([P, jbt], F32)
        bo_sb = consts.tile([1, OUT], F32)
        b_bc = consts.tile([P, OUT], F32)

        persist = top.enter_context(tc.tile_pool(name="persist", bufs=1))
        xT = persist.tile([P, KQ, N_CORE], F16, name="xT")
        ctxT = persist.tile([P, KC, J], F16, name="ctxT")
        wq_sb = persist.tile([P, KQ, INNER], F16, name="wq")
        wk_sb = persist.tile([P, KC, INNER], F16, name="wk")
        wv_sb = persist.tile([P, KC, INNER], F16, name="wv")
        wo_sb = persist.tile([P, DB, OUT], F32R, name="wo")
        qT = persist.tile([P, DB, N_CORE], F16, name="qT")
        kT = persist.tile([P, DB, J], F16, name="kT")
        vaug = persist.tile([P, jbt, HEADS, D + 1], F16, name="vaug")
        oT = persist.tile([P, DB, N_CORE], F32R, name="oT")

        ptp = top.enter_context(tc.tile_pool(name="ptp", bufs=3))
        ps_s = top.enter_context(tc.tile_pool(name="ps_s", bufs=2, space="PSUM"))
        ps_av = top.enter_context(tc.tile_pool(name="ps_av", bufs=2, space="PSUM"))
        ps_p = top.enter_context(tc.tile_pool(name="ps_p", bufs=2, space="PSUM"))
        small = top.enter_context(tc.tile_pool(name="small", bufs=2))
        outp = top.enter_context(tc.tile_pool(name="outp", bufs=3))

        # ---------- input DMAs (single queue, strict need-order) ----------
        nc.sync.dma_start(out=xT, in_=xT_d[:, :])
        nc.sync.dma_start(out=wq_sb[:, :, 0:P], in_=wq0_d[:, :])
        nc.sync.dma_start(out=ctxT, in_=ctxT_d[:, :])
        nc.sync.dma_start(out=wk_sb[:, :, 0:P], in_=wk0_d[:, :])
        nc.sync.dma_start(out=mb_sb, in_=mb_d[:, :])
        nc.sync.dma_start(out=wq_sb[:, :, P:INNER], in_=wqr_d[:, :])
        nc.sync.dma_start(out=wk_sb[:, :, P:INNER], in_=wkr_d[:, :])
        nc.sync.dma_start(out=wv_sb, in_=wv_d[:, :])
        nc.sync.dma_start(out=wo_sb, in_=wo_d[:, :].bitcast(F32R))
        nc.sync.dma_start(out=bo_sb, in_=bo_d[:, :])
        nc.gpsimd.partition_broadcast(b_bc, bo_sb)

        # ---------- projection units ----------
        def q_unit(db, ng):
            def emit():
                qp = ps_p.tile([P, 512], F32, name="pp")
                for kc in range(KQ):
                    nc.tensor.matmul(
                        qp,
                        wq_sb[:, kc, db * P:(db + 1) * P],
                        xT[:, kc, ng * 512:(ng + 1) * 512],
                        start=(kc == 0), stop=(kc == KQ - 1),
                    )
                nc.vector.tensor_copy(out=qT[:, db, ng * 512:(ng + 1) * 512], in_=qp)
            return emit

        def k_unit(db, j0, jl):
            def emit():
                kp = ps_p.tile([P, 512], F32, name="pp")
                for kc in range(KC):
                    nc.tensor.matmul(
                        kp[:, 0:jl],
                        wk_sb[:, kc, db * P:(db + 1) * P],
                        ctxT[:, kc, j0:j0 + jl],
                        start=(kc == 0), stop=(kc == KC - 1),
                    )
                nc.vector.tensor_copy(out=kT[:, db, j0:j0 + jl], in_=kp[:, 0:jl])
            return emit

        def v_unit(jb):
            def emit():
                vp = ps_p.tile([P, 512], F32, name="pp")
                for kc in range(KC):
                    nc.tensor.matmul(
                        vp,
                        ctxT[:, kc, jb * P:(jb + 1) * P],
                        wv_sb[:, kc, :],
                        start=(kc == 0), stop=(kc == KC - 1),
                    )
                nc.vector.tensor_copy(
                    out=vaug[:, jb, :, 0:D],
                    in_=vp.rearrange("p (h d) -> p h d", h=HEADS),
                )
                nc.vector.memset(vaug[:, jb, :, D:D + 1], 1.0)
            return emit

        # upfront: only what S(h0) needs; V-proj slides into head 0's loop
        # (AV(h0) runs during head 1, so all v_units are emitted before use)
        q_unit(0, 0)()
        q_unit(0, 1)()
        for (j0, jl) in jgs:
            k_unit(0, j0, jl)()

        # background units: V-proj first (popped 1/step during head 0),
        # then Q/K projections for head-pairs 1..3 (popped 1 per 3 steps)
        bg_v = [v_unit(jb) for jb in range(jbt)]
        bg = []
        for db in range(1, DB):
            bg.append(q_unit(db, 0))
            bg.append(q_unit(db, 1))
            for (j0, jl) in jgs:
                bg.append(k_unit(db, j0, jl))

        # ---------- attention ----------
        def kslice(h, jb):
            return kT[64 * (h % 2):64 * (h % 2) + 64, h // 2, jb * P:(jb + 1) * P]

        def qslice(h, ng):
            return qT[64 * (h % 2):64 * (h % 2) + 64, h // 2, ng * 512:(ng + 1) * 512]

        pts = {}

        def av_steps(h):
            """Yield closures: AV matmuls + normalize for head h (2 ng groups)."""
            for ng in range(NG):
                avp = ps_av.tile([D + 1, 512], F32, name="av")

                def mk_mm(jb, avp=avp, ng=ng):
                    def emit():
                        nc.tensor.matmul(
                            avp,
                            vaug[:, jb, h, :],
                            pts[h][:, jb, ng * 512:(ng + 1) * 512],
                            start=(jb == 0), stop=(jb == jbt - 1),
                        )
                    return emit

                for jb in range(jbt):
                    yield mk_mm(jb)

                def norm(avp=avp, ng=ng):
                    l_sb = small.tile([1, 512], F32, name="l_sb")
                    nc.vector.tensor_copy(out=l_sb, in_=avp[D:D + 1, :])
                    r_f = small.tile([1, 512], F32, name="r_f")
                    nc.vector.reciprocal_approx_fast(r_f, l_sb)
                    bc_sb = small.tile([D, 512], F32, name="bc_sb")
                    nc.gpsimd.partition_broadcast(bc_sb, r_f)
                    nc.vector.tensor_mul(
                        oT[64 * (h % 2):64 * (h % 2) + 64, h // 2,
                           ng * 512:(ng + 1) * 512],
                        avp[0:D, :],
                        bc_sb,
                    )
                yield norm

        for h in range(HEADS):
            pts[h] = ptp.tile([P, jbt, N_CORE], BF16, name="pt")
            av_it = iter(av_steps(h - 1)) if h >= 1 else None
            for jb in range(jbt):
                sp = ps_s.tile([P, N_CORE], F32, name="sp")
                for ng in range(NG):
                    nc.tensor.matmul(
                        sp[:, ng * 512:(ng + 1) * 512],
                        kslice(h, jb), qslice(h, ng),
                        start=True, stop=True,
                    )
                nc.scalar.activation(
                    out=pts[h][:, jb, :], in_=sp,
                    func=mybir.ActivationFunctionType.Exp,
                    bias=mb_sb[:, jb:jb + 1], scale=SCALE,
                )
                if av_it is not None:
                    for step in (next(av_it, None), next(av_it, None)):
                        if step is not None:
                            step()
                if bg_v:
                    bg_v.pop(0)()
                elif jb % 3 == 2 and bg:
                    bg.pop(0)()
            if av_it is not None:
                for step in av_it:
                    step()
        while bg_v:
            bg_v.pop(0)()
        while bg:
            bg.pop(0)()

        # ---------- tail: AV(h7) interleaved with out = oT^T @ Wo + b ------
        def o_group(nb, og):
            # alternate psum pools (ps_s is idle in the tail) for a deeper
            # rotation so matmul groups don't stall on the psum->sbuf add
            if (nb * 2 + og) % 2:
                op = ps_p.tile([P, 512], F32, name="pp")
            else:
                op = ps_s.tile([P, N_CORE], F32, name="sp")[:, 0:512]
            for t in range(DB):
                nc.tensor.matmul(
                    op,
                    oT[:, t, nb * P:(nb + 1) * P],
                    wo_sb[:, t, og * 512:(og + 1) * 512],
                    start=(t == 0), stop=(t == DB - 1),
                )
            ob = outp.tile([P, 512], F32, name="ob")
            nc.vector.tensor_add(ob, op, b_bc[:, og * 512:(og + 1) * 512])
            eng = nc.sync if og == 0 else nc.scalar  # drain on 2 HWDGE queues
            eng.dma_start(
                out=out_d[nb * P:(nb + 1) * P, og * 512:(og + 1) * 512],
                in_=ob,
            )

        av7 = iter(av_steps(HEADS - 1))
        for _ in range(jbt + 1):     # ng0 matmuls + norm -> oT[.., 0:512]
            next(av7)()
        rest = list(av7)             # ng1 matmuls + norm
        ri = 0
        for nb in range(NB // 2):    # out rows 0-511 ready; overlap with ng1
            for og in range(OUT // 512):
                for _ in range(2):
                    if ri < len(rest):
                        rest[ri]()
                        ri += 1
                o_group(nb, og)
        while ri < len(rest):
            rest[ri]()
            ri += 1
        for nb in range(NB // 2, NB):
            for og in range(OUT // 512):
                o_group(nb, og)

    nc.finalize()
    return nc


_NC_CACHE = {}
_LAST_JBT = 9


def _get_nc(jbt=None):
    global _LAST_JBT
    if jbt is None:
        jbt = _LAST_JBT
    _LAST_JBT = jbt
    if jbt not in _NC_CACHE:
        _NC_CACHE[jbt] = build_nc(jbt)
    return _NC_CACHE[jbt]


def make_in_maps(x, context, mask, W_q, W_k, W_v, W_o, b_o):
    global _LAST_JBT
    x = np.asarray(x, dtype=np.float32)
    context = np.asarray(context, dtype=np.float32)
    mask = np.asarray(mask).astype(bool)
    b, n, _ = x.shape
    j_full = context.shape[1]

    counts = mask.sum(axis=1)
    jbt = max(1, int(-(-int(counts.max()) // P)))  # ceil
    jbt = min(jbt, j_full // P)
    _LAST_JBT = jbt
    J = jbt * P

    def swz(a, kc):
        # [kc*128, F] -> partition-major [128, kc*F]
        f = a.shape[1]
        return np.ascontiguousarray(
            a.reshape(kc, P, f).transpose(1, 0, 2).reshape(P, kc * f)
        )

    wq4 = np.asarray(W_q, dtype=np.float16).reshape(KQ, P, INNER).transpose(1, 0, 2)
    wk4 = np.asarray(W_k, dtype=np.float16).reshape(KC, P, INNER).transpose(1, 0, 2)
    shared = {
        "wq0": np.ascontiguousarray(wq4[:, :, 0:P].reshape(P, -1)),
        "wqr": np.ascontiguousarray(wq4[:, :, P:INNER].reshape(P, -1)),
        "wk0": np.ascontiguousarray(wk4[:, :, 0:P].reshape(P, -1)),
        "wkr": np.ascontiguousarray(wk4[:, :, P:INNER].reshape(P, -1)),
        "wv": swz(np.asarray(W_v, dtype=np.float16), KC),
        "wo": swz(np.asarray(W_o, dtype=np.float32), DB),
        "bo": np.ascontiguousarray(
            np.asarray(b_o, dtype=np.float32).reshape(1, OUT)
        ),
    }
    # per-batch: pack unmasked context rows first (order-preserving), truncate
    # to J (dropped rows are all masked => contribute exactly 0), transpose.
    ctxT_b, mb_b = [], []
    for bi in range(b):
        idx = np.argsort(~mask[bi], kind="stable")[:J]
        ctxp = context[bi][idx]
        mkp = mask[bi][idx]
        ctxT_b.append(swz(np.ascontiguousarray(ctxp.T.astype(np.float16)), KC))
        mb_b.append(np.ascontiguousarray(
            np.where(mkp, 0.0, MASK_NEG).astype(np.float32).reshape(jbt, P).T
        ))

    in_maps = []
    for c in range(8):
        bi, nh = c // 2, c % 2
        xT_c = swz(np.ascontiguousarray(
            x[bi, nh * N_CORE:(nh + 1) * N_CORE].T.astype(np.float16)
        ), KQ)
        in_maps.append({
            "x": xT_c,
            "ctx": ctxT_b[bi],
            "mb": mb_b[bi],
            **shared,
        })
    return in_maps


def kernel(x, context, mask, W_q, W_k, W_v, W_o, b_o):
    in_maps = make_in_maps(x, context, mask, W_q, W_k, W_v, W_o, b_o)
    nc = _get_nc(_LAST_JBT)
    res = run_bass_kernel_spmd(nc, in_maps, core_ids=list(range(8)))
    out = np.empty((4, 2048, OUT), dtype=np.float32)
    for c in range(8):
        bi, nh = c // 2, c % 2
        out[bi, nh * N_CORE:(nh + 1) * N_CORE] = res.results[c]["out"]
    return out
